# revision 24
# baseline (speedup 1.0000x reference)
"""AMFormer layer on 8 Trainium2 NeuronCores.

Sharding: data-parallel over batch (16 batches -> 2 per core), zero
collectives.  All matmuls run as float32r (11-bit mantissa, fp32
accumulate, 1 PE cycle/row).  LayerNorms are folded into the following
projections via two augmented contraction rows (-mu, 1/rstd) whose
weight-side rows (column sums u, folded bias b') are precomputed on the
host.  add/mul attention heads are interleaved in the combined Q/K
weights; each k-chunk's two score matmuls write the two halves of one
2-bank PSUM tile so a single ACT exp evacuates both.  V projections are
emitted token-major in a 65-column-per-head layout whose 65th column is
a bias-produced constant 1.0, so the attention-value matmuls also
produce softmax denominators for free.  The multiplicative branch's
sign() threshold uses sign(sign_w + EPS*sumexp) == sign(sign_w/sumexp +
EPS), accumulated into the same PSUM tile with one rank-1 matmul.
Attention is software-pipelined one head deep (scores/exp of head h+1
issue before the AV matmuls of head h) to keep the in-order PE stream
from stalling on ACT.  LN2 + FFN run per 512-token batch so their dense
matmuls can overlap the other batch's attention phase.
"""

import os
import sys
import types
import contextlib
import numpy as np

# ---------------------------------------------------------------- constants
B, S, D = 16, 512, 768
H, DH = 12, 64
DFF = 4 * D
N_CORES = 8
BPC = B // N_CORES          # batches per core
TOK = BPC * S               # tokens per core (1024)
NTC = TOK // 128            # token chunks per core (8)
SCALE = float(np.sqrt(DH))
EPS = 1e-6
LN_EPS = 1e-5
DHA = DH + 1                # augmented head width in V layout (65)
VW = H * DHA                # augmented V width per branch (780)
KC = D // 128               # feature chunks (6)
FC = 2 * D // 128           # combined q/k output chunks (12)
FFC = DFF // 128            # ffn hidden chunks (24)

_f32 = np.float32


def _f32r(a):
    """Round fp32 -> float32r bit pattern (11 mantissa bits, round-nearest)."""
    a = np.ascontiguousarray(a, _f32)
    u = a.view(np.uint32).copy()
    u = (u + np.uint32(0x800)) & np.uint32(0xFFFFF000)
    return u.view(np.float32)


# ------------------------------------------------------------- axon shims
def _install_shims():
    if "antenv.axon_hooks" not in sys.modules:
        try:
            import trn_agent_boot.trn_boot as tb
            hook = tb._ntff_profile_via_ctypes("/opt/axon/libaxon_pjrt.so")
        except Exception:
            hook = None
        mod = types.ModuleType("antenv.axon_hooks")
        mod.get_axon_ntff_profile_hook = lambda: hook
        mod.set_axon_ntff_profile_hook = lambda h: None
        sys.modules["antenv.axon_hooks"] = mod
    try:
        import concourse.bass_utils as bu
        bu.upload_artifacts = lambda tmpdir: f"local:{tmpdir}"
    except Exception:
        pass


# ------------------------------------------------- walrus sync-wait limiter
def _split_sync_waits(nc, max_waits=1):
    """This container's walrus accepts a single sync-wait per instruction;
    move extras onto same-engine NOPs placed immediately before."""
    import bass_rust
    from concourse import mybir

    for f in nc.m.functions:
        for bb in f.blocks:
            out = []
            for inst in bb.instructions:
                si = inst.sync_info
                if si is not None and si.on_wait and len(si.on_wait) > max_waits:
                    waits = list(si.on_wait)
                    extra, keep = waits[:-max_waits], waits[-max_waits:]
                    for i in range(0, len(extra), max_waits):
                        nop = mybir.InstNoOp(
                            name=f"I-splitwait-{nc.next_id()}",
                            engine=inst.engine,
                            sync_info=bass_rust.SyncInfo(
                                on_wait=extra[i : i + max_waits], on_update=[]
                            ),
                        )
                        nc.register_instruction(nop)
                        out.append(nop)
                    si.on_wait = keep
                out.append(inst)
            bb.instructions[:] = out


# --------------------------------------------------------------- host prep
def _prep_weights(p):
    g1, b1 = np.asarray(p["ln1_g"], _f32), np.asarray(p["ln1_b"], _f32)
    g2, b2 = np.asarray(p["ln2_g"], _f32), np.asarray(p["ln2_b"], _f32)
    alpha = _f32(1.0 / (1.0 + np.exp(-np.float64(np.asarray(p["mix_weight"])))))

    def fold_ln(W, b, g, bln):
        W = np.asarray(W, _f32)
        return W * g[None, :], W @ bln + np.asarray(b, _f32)

    def aug_T(Wf, bf):
        u = Wf.sum(axis=1)
        return np.concatenate([Wf.T, u[None, :], bf[None, :]], axis=0).astype(_f32)

    def qk_combined(name, scale):
        Wa, ba = fold_ln(p[f"add_{name}_w"], p[f"add_{name}_b"], g1, b1)
        Wm, bm = fold_ln(p[f"mul_{name}_w"], p[f"mul_{name}_b"], g1, b1)
        Wa, ba, Wm, bm = Wa * scale, ba * scale, Wm * scale, bm * scale
        W = np.zeros((2 * D, D), _f32)
        b = np.zeros((2 * D,), _f32)
        for h in range(H):
            W[h * 128 : h * 128 + 64] = Wa[h * 64 : (h + 1) * 64]
            W[h * 128 + 64 : (h + 1) * 128] = Wm[h * 64 : (h + 1) * 64]
            b[h * 128 : h * 128 + 64] = ba[h * 64 : (h + 1) * 64]
            b[h * 128 + 64 : (h + 1) * 128] = bm[h * 64 : (h + 1) * 64]
        return aug_T(W, b)  # [770, 1536]

    Wq = qk_combined("q", _f32(1.0 / SCALE))
    Wk = qk_combined("k", _f32(1.0))

    def v_aug(prefix):
        Wf, bf = fold_ln(p[f"{prefix}_v_w"], p[f"{prefix}_v_b"], g1, b1)
        W = np.zeros((VW, D), _f32)
        b = np.zeros((VW,), _f32)
        for h in range(H):
            W[h * DHA : h * DHA + DH] = Wf[h * DH : (h + 1) * DH]
            b[h * DHA : h * DHA + DH] = bf[h * DH : (h + 1) * DH]
            b[h * DHA + DH] = 1.0  # ones column via bias row
        return W, b

    Wva, bva = v_aug("add")
    Wvm, bvm = v_aug("mul")
    Wv = aug_T(np.concatenate([Wva, Wvm], 0), np.concatenate([bva, bvm], 0))

    Woa = alpha * np.asarray(p["add_o_w"], _f32)
    boa = alpha * np.asarray(p["add_o_b"], _f32)
    Wom = (1.0 - alpha) * np.asarray(p["mul_o_w"], _f32)
    bom = (1.0 - alpha) * np.asarray(p["mul_o_b"], _f32)
    Wo = np.concatenate([Woa.T, Wom.T, (boa + bom)[None, :]], 0).astype(_f32)

    W1 = aug_T(*fold_ln(p["ffn1_w"], p["ffn1_b"], g2, b2))
    W2 = np.concatenate(
        [np.asarray(p["ffn2_w"], _f32).T, np.asarray(p["ffn2_b"], _f32)[None, :]], 0
    )

    return {
        "wq": _f32r(Wq), "wk": _f32r(Wk), "wv": _f32r(Wv),
        "wo": _f32r(Wo), "w1": _f32r(W1), "w2": _f32r(W2),
    }


# ------------------------------------------------------------ device build
_BUILD_CACHE = {}


def _build():
    if "nc" in _BUILD_CACHE:
        return _BUILD_CACHE["nc"]
    import concourse.bass as bass
    import concourse.tile as tile
    from concourse import mybir
    from concourse.masks import make_identity

    dt = mybir.dt
    F32, F32R = dt.float32, dt.float32r
    AF = mybir.ActivationFunctionType
    ALU = mybir.AluOpType

    nc = bass.Bass("TRN2", target_bir_lowering=False, num_devices=N_CORES)

    def act_recip(out_ap, in_ap):
        """ACT Reciprocal (spline) — ~1e-5 rel err, 1 op.  Built directly:
        the bass wrapper refuses Reciprocal for precision reasons that do
        not matter at our tolerance."""
        ins = [
            nc.scalar.lower_ap(in_ap),
            mybir.ImmediateValue(dtype=F32, value=0.0),
            mybir.ImmediateValue(dtype=F32, value=1.0),
            mybir.ImmediateValue(dtype=F32, value=0.0),
        ]
        nc.scalar.add_instruction(
            mybir.InstActivation(
                name=f"I-{nc.next_id()}",
                func=AF.Reciprocal,
                ins=ins,
                outs=[nc.scalar.lower_ap(out_ap)],
            )
        )

    x_ext = nc.declare_dram_parameter("x", [TOK, D], F32, isOutput=False)
    wq_ext = nc.declare_dram_parameter("wq", [D + 2, 2 * D], F32R, isOutput=False)
    wk_ext = nc.declare_dram_parameter("wk", [D + 2, 2 * D], F32R, isOutput=False)
    wv_ext = nc.declare_dram_parameter("wv", [D + 2, 2 * VW], F32R, isOutput=False)
    wo_ext = nc.declare_dram_parameter("wo", [2 * D + 1, D], F32R, isOutput=False)
    w1_ext = nc.declare_dram_parameter("w1", [D + 2, DFF], F32R, isOutput=False)
    w2_ext = nc.declare_dram_parameter("w2", [DFF + 1, D], F32R, isOutput=False)
    out_ext = nc.declare_dram_parameter("out", [TOK, D], F32, isOutput=True)

    x_tiled = x_ext.ap().rearrange("(n p) d -> p n d", p=128)
    out_tiled = out_ext.ap().rearrange("(n p) d -> p n d", p=128)

    with contextlib.ExitStack() as top:
        tc = top.enter_context(tile.TileContext(nc))

        const_pool = top.enter_context(tc.tile_pool(name="const", bufs=1))
        persist = top.enter_context(tc.tile_pool(name="persist", bufs=1))
        arena = top.enter_context(tc.tile_pool(name="arena", bufs=1))
        lnrows = top.enter_context(tc.tile_pool(name="lnrows", bufs=1))
        wpool = top.enter_context(tc.tile_pool(name="wts", bufs=5))

        ident = const_pool.tile([128, 128], F32)
        make_identity(nc, ident[:])
        # f32r constants: memset fp32 staging, ACT-copy to f32r (direct
        # f32r memset fails walrus ISA validation)
        stage = const_pool.tile([128, TOK], F32, name="stage")
        nc.vector.memset(stage[:], 1.0)
        ones_col = const_pool.tile([128, 1], F32R)
        nc.scalar.copy(ones_col[:], stage[:, 0:1])
        ones_row = const_pool.tile([1, TOK], F32R)
        nc.scalar.copy(ones_row[:], stage[0:1, :])
        eps_row = const_pool.tile([1, DH], F32R)
        nc.scalar.activation(eps_row[:], stage[0:1, 0:DH], AF.Copy, scale=EPS)
        eps_col = const_pool.tile([128, 1], F32)
        nc.vector.memset(eps_col[:], EPS)
        lneps_col = const_pool.tile([1, 1], F32)
        nc.vector.memset(lneps_col[:], LN_EPS)

        # persistent feature-major tensor (f32r) with LN aug rows in chunk
        # KC; x2 = x + attention overwrites it in place (disjoint per-batch
        # column slices)
        xaug = persist.tile([128, KC + 1, TOK], F32R, tag="xaug")
        x2aug = xaug
        rstd1_bc = persist.tile([128, TOK], F32, tag="rstd1_bc")
        rstd2_bc = persist.tile([128, TOK], F32, tag="rstd2_bc")
        rstd1_cols = lnrows.tile([128, NTC], F32, tag="rstd1_cols")
        rstd2_cols = lnrows.tile([128, NTC], F32, tag="rstd2_cols")

        def transpose_fm(dst, src_view):
            """token-major [128, NTC, 768] -> feature-major dst chunks."""
            with tc.tile_pool(name="trp", bufs=6, space="PSUM") as trp:
                for c in range(KC):
                    for t in range(NTC):
                        pt = trp.tile([128, 128], F32, tag="tr")
                        nc.tensor.transpose(
                            pt[:], src_view[:, t, c * 128 : (c + 1) * 128], ident[:]
                        )
                        nc.scalar.copy(dst[:, c, t * 128 : (t + 1) * 128], pt[:])

        def ln_stats(src, rstd_bc, rstd_cols, sq, w0, nw):
            """LN stats over token window [w0, w0+nw): write -mu / 1/rstd
            into src chunk-KC rows 0/1 (window slice), fill rstd_bc window
            and per-128-chunk rstd columns."""
            wsl = slice(w0, w0 + nw)
            with (
                tc.tile_pool(name="lnsb", bufs=1) as lnsb,
                tc.tile_pool(name="lnr", bufs=2) as lnr,
                tc.tile_pool(name="lnps", bufs=1, space="PSUM") as lnps,
            ):
                for c in range(KC):
                    nc.scalar.activation(sq[:, c, 0:nw], src[:, c, wsl], AF.Square)
                sum_x = lnsb.tile([1, TOK], F32, tag="sum_x")
                sum_q = lnsb.tile([1, TOK], F32, tag="sum_q")
                for half in range(nw // 512):
                    hs = slice(half * 512, (half + 1) * 512)
                    gs = slice(w0 + half * 512, w0 + (half + 1) * 512)
                    psx = lnps.tile([1, 512], F32, tag="stx")
                    psq = lnps.tile([1, 512], F32, tag="stq")
                    for c in range(KC):
                        nc.tensor.matmul(
                            psx[:], ones_col[:], src[:, c, gs],
                            start=(c == 0), stop=(c == KC - 1),
                        )
                    for c in range(KC):
                        nc.tensor.matmul(
                            psq[:], ones_col[:], sq[:, c, hs],
                            start=(c == 0), stop=(c == KC - 1),
                        )
                    nc.vector.tensor_copy(sum_x[:, hs], psx[:])
                    nc.vector.tensor_copy(sum_q[:, hs], psq[:])
                nc.scalar.activation(
                    src[0:1, KC, wsl], sum_x[:, 0:nw], AF.Copy, scale=-1.0 / D
                )
                mu2 = lnr.tile([1, TOK], F32, tag="mu2")
                nc.scalar.activation(
                    mu2[:, 0:nw], sum_x[:, 0:nw], AF.Square, scale=1.0 / D
                )
                var = lnr.tile([1, TOK], F32, tag="var")
                nc.vector.tensor_scalar(
                    out=var[:, 0:nw], in0=sum_q[:, 0:nw], scalar1=1.0 / D,
                    scalar2=None, op0=ALU.mult,
                )
                nc.vector.tensor_sub(var[:, 0:nw], var[:, 0:nw], mu2[:, 0:nw])
                lnv = lnr.tile([1, TOK], F32, tag="lnv")
                nc.scalar.activation(
                    lnv[:, 0:nw], var[:, 0:nw], AF.Ln, bias=lneps_col[:]
                )
                rstd_row = lnr.tile([1, TOK], F32R, tag="rstd_row")
                nc.scalar.activation(
                    rstd_row[:, 0:nw], lnv[:, 0:nw], AF.Exp, scale=-0.5
                )
                rstd_inv = lnr.tile([1, TOK], F32R, tag="rstd_inv")
                nc.scalar.activation(
                    rstd_inv[:, 0:nw], lnv[:, 0:nw], AF.Exp, scale=0.5
                )
                # partition-1 writes need the DMA path (compute APs must be
                # 32-aligned in partition base)
                nc.sync.dma_start(out=src[1:2, KC, wsl], in_=rstd_inv[:, 0:nw])
                for half in range(nw // 512):
                    hs = slice(half * 512, (half + 1) * 512)
                    gs = slice(w0 + half * 512, w0 + (half + 1) * 512)
                    pb = lnps.tile([128, 512], F32, tag="rb")
                    nc.tensor.matmul(
                        pb[:], ones_row[0:1, 0:128], rstd_row[:, hs],
                        start=True, stop=True,
                    )
                    nc.scalar.copy(rstd_bc[:, gs], pb[:])
                # row -> per-chunk columns via tiny SBUF DMAs
                for t in range(nw // 128):
                    nc.sync.dma_start(
                        out=rstd_cols[:, w0 // 128 + t : w0 // 128 + t + 1],
                        in_=rstd_row[0:1, t * 128 : (t + 1) * 128].bitcast(F32),
                    )

        # ---------------- phase 0: load x, transpose, LN1 stats
        x_tm = arena.tile([128, NTC, D], F32, tag="bigA", name="x_tm")
        for t in range(NTC):
            nc.sync.dma_start(out=x_tm[:, t, :], in_=x_tiled[:, t, :])
        transpose_fm(xaug, x_tm)
        sq1 = arena.tile([128, KC, TOK], F32R, tag="bigB", name="sq1")
        ln_stats(xaug, rstd1_bc, rstd1_cols, sq1, 0, TOK)

        def load_w(wext, c, cols, kpart=128, tag="wt"):
            wt = wpool.tile([128, 512], F32R, tag=tag, name=f"w_{tag}")
            nc.sync.dma_start(
                out=wt[0:kpart, 0 : cols.stop - cols.start],
                in_=wext.ap()[c * 128 : c * 128 + kpart, cols],
            )
            return wt

        # ============ per-batch: QKV -> attention -> O-proj -> FFN ========
        for b in range(BPC):
            tsl = slice(b * 512, (b + 1) * 512)

            with contextlib.ExitStack() as bctx:
                bpool = bctx.enter_context(tc.tile_pool(name=f"bt{b}", bufs=1))
                q_sb = arena.tile([128, FC, 512], F32R, tag="bigA",
                                  name=f"q_sb{b}")
                k_sb = arena.tile([128, FC, 512], F32R, tag="bigB",
                                  name=f"k_sb{b}")

                # ---- q/k projections (feature-major, oc groups of 4)
                with tc.tile_pool(name="pp", bufs=5, space="PSUM") as ppq:
                    for wext, dst in ((wq_ext, q_sb), (wk_ext, k_sb)):
                        for og in range(FC // 4):
                            pss = [
                                ppq.tile([128, 512], F32, tag="pp",
                                         name=f"pp{_j}")
                                for _j in range(4)
                            ]
                            for c in range(KC + 1):
                                kpart = 128 if c < KC else 2
                                wt = load_w(
                                    wext, c,
                                    slice(og * 512, (og + 1) * 512), kpart,
                                )
                                for j in range(4):
                                    nc.tensor.matmul(
                                        pss[j][:],
                                        wt[0:kpart, j * 128 : (j + 1) * 128],
                                        xaug[0:kpart, min(c, KC), tsl],
                                        start=(c == 0), stop=(c == KC),
                                    )
                            for j in range(4):
                                nc.vector.tensor_mul(
                                    dst[:, og * 4 + j, :], pss[j][:],
                                    rstd1_bc[:, tsl],
                                )

                    # ---- v projections (token-major)
                    v_add = bpool.tile([128, 4, VW], F32R, tag="v_add")
                    v_mul = bpool.tile([128, 4, VW], F32R, tag="v_mul")
                    v_sgn = bpool.tile([128, 4, VW], F32R, tag="v_sgn")
                    vdo = [(0, 512), (512, VW - 512), (VW, 512),
                           (VW + 512, VW - 512)]
                    with tc.tile_pool(name="wvp", bufs=7) as wvp:
                        for o0, ow in vdo:
                            wts = []
                            for c in range(KC + 1):
                                kpart = 128 if c < KC else 2
                                wt = wvp.tile([128, 512], F32R, tag="wv",
                                              name="wv_t")
                                nc.sync.dma_start(
                                    out=wt[0:kpart, 0:ow],
                                    in_=wv_ext.ap()[c * 128 : c * 128 + kpart,
                                                    o0 : o0 + ow],
                                )
                                wts.append(wt)
                            for t in range(4):
                                gt = b * 4 + t
                                ps = ppq.tile([128, 512], F32, tag="pp",
                                              name="pv")
                                for c in range(KC + 1):
                                    kpart = 128 if c < KC else 2
                                    nc.tensor.matmul(
                                        ps[:, 0:ow],
                                        xaug[0:kpart, min(c, KC),
                                             gt * 128 : (gt + 1) * 128],
                                        wts[c][0:kpart, 0:ow],
                                        start=(c == 0), stop=(c == KC),
                                    )
                                dst = v_add if o0 < VW else v_mul
                                d0 = o0 if o0 < VW else o0 - VW
                                nc.scalar.activation(
                                    dst[:, t, d0 : d0 + ow], ps[:, 0:ow],
                                    AF.Copy,
                                    scale=rstd1_cols[:, gt : gt + 1],
                                )

                # sign / log(|.|+eps) on the 64-col head blocks of v_mul
                for t in range(4):
                    vm = v_mul[:, t, :].rearrange("p (h w) -> p h w", h=H)[
                        :, :, 0:DH]
                    vs = v_sgn[:, t, :].rearrange("p (h w) -> p h w", h=H)[
                        :, :, 0:DH]
                    nc.scalar.activation(vs, vm, AF.Sign)
                    nc.scalar.activation(vm, vm, AF.Abs)
                    nc.scalar.activation(vm, vm, AF.Ln, bias=eps_col[:])

                # ---- attention (1-head-deep software pipeline)
                av_stack = bpool.tile([128, FC, 512], F32R, tag="av_stack")
                with (
                    tc.tile_pool(name="att", bufs=4) as att,
                    tc.tile_pool(name="att1", bufs=1) as att1,
                    tc.tile_pool(name="attr", bufs=1) as attr,
                    tc.tile_pool(name="aps", bufs=2, space="PSUM") as aps,
                    tc.tile_pool(name="avps", bufs=1, space="PSUM") as avps,
                    tc.tile_pool(name="rbps", bufs=1, space="PSUM") as rbps,
                ):
                    e_pairs = {}

                    def scores_exp(h):
                        es = []
                        for kc4 in range(4):
                            ks = slice(kc4 * 128, (kc4 + 1) * 128)
                            psp = aps.tile([128, 1024], F32, tag="ss",
                                           name=f"ss{h}_{kc4}")
                            nc.tensor.matmul(
                                psp[:, 0:512], k_sb[0:64, h, ks],
                                q_sb[0:64, h, :],
                                start=True, stop=True, tile_position=(0, 0),
                            )
                            nc.tensor.matmul(
                                psp[:, 512:1024], k_sb[64:128, h, ks],
                                q_sb[64:128, h, :],
                                start=True, stop=True, tile_position=(64, 0),
                            )
                            e = att.tile([128, 1024], F32R, tag="e_pair",
                                         name=f"e{h}_{kc4}")
                            nc.scalar.activation(e[:], psp[:], AF.Exp)
                            es.append(e)
                        e_pairs[h] = es

                    def av_epilogue(h):
                        es = e_pairs.pop(h)
                        pa = avps.tile([DHA, 512], F32, tag="pa", name=f"pa{h}")
                        pl = avps.tile([DHA, 512], F32, tag="pl", name=f"pl{h}")
                        pg = avps.tile([DH, 512], F32, tag="pg", name=f"pg{h}")
                        for kc4 in range(4):
                            ea = es[kc4][:, 0:512]
                            em = es[kc4][:, 512:1024]
                            nc.tensor.matmul(
                                pa[:], v_add[:, kc4, h * DHA : h * DHA + DHA],
                                ea, start=(kc4 == 0), stop=(kc4 == 3),
                            )
                            nc.tensor.matmul(
                                pl[:], v_mul[:, kc4, h * DHA : h * DHA + DHA],
                                em, start=(kc4 == 0), stop=(kc4 == 3),
                            )
                            nc.tensor.matmul(
                                pg[:], v_sgn[:, kc4, h * DHA : h * DHA + DH],
                                em, start=(kc4 == 0), stop=False,
                            )
                        # evacuate immediately: the epilogue chain below then
                        # reads SBUF, releasing pa/pl for the next head
                        av_u = att.tile([DHA, 1024], F32, tag="av_u",
                                        name=f"avu{h}", bufs=2)
                        nc.vector.tensor_copy(av_u[:, 0:512], pa[:])
                        nc.vector.tensor_copy(av_u[:, 512:1024], pl[:])
                        s_mul = attr.tile([1, 512], F32R, tag="s_mul",
                                          name=f"sm{h}")
                        nc.vector.tensor_copy(
                            s_mul[:], av_u[DH : DH + 1, 512:1024]
                        )
                        nc.tensor.matmul(
                            pg[:], eps_row[:], s_mul[:], start=False, stop=True
                        )
                        sgn = att1.tile([DH, 512], F32R, tag="sgn")
                        nc.scalar.activation(sgn[:], pg[:], AF.Sign)
                        # add-branch normalize: 1/S = exp(-ln(S))
                        lna = attr.tile([1, 512], F32, tag="lna",
                                        name=f"lna{h}")
                        nc.scalar.activation(
                            lna[:], av_u[DH : DH + 1, 0:512], AF.Ln
                        )
                        raf = attr.tile([1, 512], F32R, tag="raf",
                                        name=f"raf{h}")
                        nc.scalar.activation(raf[:], lna[:], AF.Exp, scale=-1.0)
                        prb = rbps.tile([DH, 512], F32, tag="prb",
                                        name=f"prb{h}")
                        nc.tensor.matmul(
                            prb[:], ones_row[0:1, 0:DH], raf[:],
                            start=True, stop=True,
                        )
                        rab = att1.tile([DH, 512], F32, tag="rab")
                        nc.vector.tensor_copy(rab[:], prb[:])
                        nc.vector.tensor_mul(
                            av_stack[(h % 2) * 64 : (h % 2) * 64 + 64,
                                     h // 2, :],
                            av_u[0:DH, 0:512], rab[:],
                        )
                        # mul-branch normalize + sign
                        lnm = attr.tile([1, 512], F32, tag="lnm",
                                        name=f"lnm{h}")
                        nc.scalar.activation(
                            lnm[:], av_u[DH : DH + 1, 512:1024], AF.Ln
                        )
                        rmf = attr.tile([1, 512], F32R, tag="rmf",
                                        name=f"rmf{h}")
                        nc.scalar.activation(rmf[:], lnm[:], AF.Exp, scale=-1.0)
                        pmb = rbps.tile([DH, 512], F32, tag="prb",
                                        name=f"pmb{h}")
                        nc.tensor.matmul(
                            pmb[:], ones_row[0:1, 0:DH], rmf[:],
                            start=True, stop=True,
                        )
                        rmb = att1.tile([DH, 512], F32, tag="rmb")
                        nc.vector.tensor_copy(rmb[:], pmb[:])
                        lon = att1.tile([DH, 512], F32, tag="lon")
                        nc.vector.tensor_mul(lon[:], av_u[0:DH, 512:1024],
                                             rmb[:])
                        elo = att1.tile([DH, 512], F32R, tag="elo")
                        nc.scalar.activation(elo[:], lon[:], AF.Exp)
                        nc.gpsimd.tensor_mul(
                            av_stack[(h % 2) * 64 : (h % 2) * 64 + 64,
                                     6 + h // 2, :],
                            elo[:], sgn[:],
                        )

                    for step in range(H + 1):
                        if step < H:
                            scores_exp(step)
                        if step >= 1:
                            av_epilogue(step - 1)

                # ---- O-projection + mix + residual -> x2 (feature-major)
                with tc.tile_pool(name="ops", bufs=5, space="PSUM") as ops:
                    for og in range(2):
                        n_out = 4 if og == 0 else 2
                        pss = [
                            ops.tile([128, 512], F32, tag="po",
                                     name=f"po{_j}")
                            for _j in range(n_out)
                        ]
                        for c in range(FC + 1):
                            kpart = 128 if c < FC else 1
                            wt = load_w(
                                wo_ext, c,
                                slice(og * 512, og * 512 + n_out * 128), kpart,
                            )
                            for j in range(n_out):
                                src = (
                                    av_stack[:, c, :] if c < FC
                                    else ones_row[:, 0:512]
                                )
                                nc.tensor.matmul(
                                    pss[j][0:128, :],
                                    wt[0:kpart, j * 128 : (j + 1) * 128],
                                    src[0:kpart, :] if c < FC else src,
                                    start=(c == 0), stop=(c == FC),
                                )
                        for j in range(n_out):
                            oc = og * 4 + j
                            nc.vector.tensor_add(
                                x2aug[:, oc, tsl], pss[j][:], xaug[:, oc, tsl]
                            )


        # ================ LN2 stats (both batches) + FFN per half =========
        sq2 = arena.tile([128, KC, TOK], F32R, tag="bigB", name="sq2")
        ln_stats(x2aug, rstd2_bc, rstd2_cols, sq2, 0, TOK)

        with (
            tc.tile_pool(name="gsb", bufs=1) as gsb,
            tc.tile_pool(name="fwork", bufs=3) as fwork,
            tc.tile_pool(name="orow", bufs=2) as orow,
            tc.tile_pool(name="fps", bufs=5, space="PSUM") as fps,
            tc.tile_pool(name="trp2", bufs=3, space="PSUM") as trp2,
        ):
            for b in range(BPC):
                hsl = slice(b * 512, (b + 1) * 512)
                g_sb = gsb.tile([128, FFC, 512], F32R, tag="g_sb",
                                name=f"g_sb{b}")
                for og in range(FFC // 4):
                    pss = [fps.tile([128, 512], F32, tag="pf", name=f"pf{_j}")
                           for _j in range(4)]
                    for c in range(KC + 1):
                        kpart = 128 if c < KC else 2
                        wt = load_w(
                            w1_ext, c, slice(og * 512, (og + 1) * 512), kpart,
                        )
                        for j in range(4):
                            nc.tensor.matmul(
                                pss[j][:],
                                wt[0:kpart, j * 128 : (j + 1) * 128],
                                x2aug[0:kpart, min(c, KC), hsl],
                                start=(c == 0), stop=(c == KC),
                            )
                    for j in range(4):
                        pre = fwork.tile([128, 512], F32, tag="pre")
                        nc.vector.tensor_mul(pre[:], pss[j][:], rstd2_bc[:, hsl])
                        nc.scalar.activation(
                            g_sb[:, og * 4 + j, :], pre[:], AF.Gelu
                        )

                # FFN2 + residual -> out (feature-major, fp32)
                out_fm = gsb.tile([128, KC, 512], F32, tag="out_fm",
                                  name=f"out_fm{b}")
                for og in range(2):
                    n_out = 4 if og == 0 else 2
                    pss = [
                        fps.tile([128, 512], F32, tag="pf", name=f"pf2{_j}")
                        for _j in range(n_out)
                    ]
                    for c in range(FFC + 1):
                        kpart = 128 if c < FFC else 1
                        wt = load_w(
                            w2_ext, c,
                            slice(og * 512, og * 512 + n_out * 128), kpart,
                        )
                        for j in range(n_out):
                            src = g_sb[:, c, :] if c < FFC else ones_row[:, hsl]
                            nc.tensor.matmul(
                                pss[j][:],
                                wt[0:kpart, j * 128 : (j + 1) * 128],
                                src,
                                start=(c == 0), stop=(c == FFC),
                            )
                    for j in range(n_out):
                        oc = og * 4 + j
                        nc.vector.tensor_add(
                            out_fm[:, oc, :], pss[j][:], x2aug[:, oc, hsl]
                        )

                # transpose to token-major, one DMA per 128-token row block
                for t in range(4):
                    row = orow.tile([128, D], F32, tag="row")
                    for c in range(KC):
                        pt = trp2.tile([128, 128], F32, tag="tr2")
                        nc.tensor.transpose(
                            pt[:], out_fm[:, c, t * 128 : (t + 1) * 128],
                            ident[:],
                        )
                        nc.scalar.copy(row[:, c * 128 : (c + 1) * 128], pt[:])
                    nc.sync.dma_start(
                        out=out_tiled[:, b * 4 + t, :], in_=row[:]
                    )

    _split_sync_waits(nc, max_waits=1)
    _BUILD_CACHE["nc"] = nc
    return nc


# ---------------------------------------------------------------- kernel()
LAST_EXEC_TIME_NS = None


def kernel(x, params):
    global LAST_EXEC_TIME_NS
    _install_shims()
    from concourse.bass_utils import run_bass_kernel_spmd

    x = np.ascontiguousarray(np.asarray(x, _f32))
    prep = _prep_weights(params)
    nc = _build()

    in_maps = []
    for c in range(N_CORES):
        shard = x[c * BPC : (c + 1) * BPC].reshape(TOK, D)
        in_maps.append({"x": np.ascontiguousarray(shard), **prep})

    trace = bool(int(os.environ.get("KBENCH_TRACE", "0")))
    res = run_bass_kernel_spmd(nc, in_maps, list(range(N_CORES)), trace=trace)
    LAST_EXEC_TIME_NS = res.exec_time_ns

    out = np.empty((B, S, D), _f32)
    for c in range(N_CORES):
        out[c * BPC : (c + 1) * BPC] = res.results[c]["out"].reshape(BPC, S, D)
    return out


# revision 26
# speedup vs baseline: 1.0337x; 1.0337x over previous
"""AMFormer layer on 8 Trainium2 NeuronCores.

Sharding: data-parallel over batch (16 batches -> 2 per core), zero
collectives.  All matmuls run as float32r (11-bit mantissa, fp32
accumulate, 1 PE cycle/row).  LayerNorms are folded into the following
projections via two augmented contraction rows (-mu, 1/rstd) whose
weight-side rows (column sums u, folded bias b') are precomputed on the
host.  add/mul attention heads are interleaved in the combined Q/K
weights; each k-chunk's two score matmuls write the two halves of one
2-bank PSUM tile so a single ACT exp evacuates both.  V projections are
emitted token-major in a 65-column-per-head layout whose 65th column is
a bias-produced constant 1.0, so the attention-value matmuls also
produce softmax denominators for free.  The multiplicative branch's
sign() threshold uses sign(sign_w + EPS*sumexp) == sign(sign_w/sumexp +
EPS), accumulated into the same PSUM tile with one rank-1 matmul.
Attention is software-pipelined one head deep (scores/exp of head h+1
issue before the AV matmuls of head h) to keep the in-order PE stream
from stalling on ACT.  LN2 + FFN run per 512-token batch so their dense
matmuls can overlap the other batch's attention phase.
"""

import os
import sys
import types
import contextlib
import numpy as np

# ---------------------------------------------------------------- constants
B, S, D = 16, 512, 768
H, DH = 12, 64
DFF = 4 * D
N_CORES = 8
BPC = B // N_CORES          # batches per core
TOK = BPC * S               # tokens per core (1024)
NTC = TOK // 128            # token chunks per core (8)
SCALE = float(np.sqrt(DH))
EPS = 1e-6
LN_EPS = 1e-5
DHA = DH + 1                # augmented head width in V layout (65)
VW = H * DHA                # augmented V width per branch (780)
KC = D // 128               # feature chunks (6)
FC = 2 * D // 128           # combined q/k output chunks (12)
FFC = DFF // 128            # ffn hidden chunks (24)

_f32 = np.float32


def _f32r(a):
    """Round fp32 -> float32r bit pattern (11 mantissa bits, round-nearest)."""
    a = np.ascontiguousarray(a, _f32)
    u = a.view(np.uint32).copy()
    u = (u + np.uint32(0x800)) & np.uint32(0xFFFFF000)
    return u.view(np.float32)


# ------------------------------------------------------------- axon shims
def _install_shims():
    if "antenv.axon_hooks" not in sys.modules:
        try:
            import trn_agent_boot.trn_boot as tb
            hook = tb._ntff_profile_via_ctypes("/opt/axon/libaxon_pjrt.so")
        except Exception:
            hook = None
        mod = types.ModuleType("antenv.axon_hooks")
        mod.get_axon_ntff_profile_hook = lambda: hook
        mod.set_axon_ntff_profile_hook = lambda h: None
        sys.modules["antenv.axon_hooks"] = mod
    try:
        import concourse.bass_utils as bu
        bu.upload_artifacts = lambda tmpdir: f"local:{tmpdir}"
    except Exception:
        pass


# ------------------------------------------------- walrus sync-wait limiter
def _split_sync_waits(nc, max_waits=1):
    """This container's walrus accepts a single sync-wait per instruction;
    move extras onto same-engine NOPs placed immediately before."""
    import bass_rust
    from concourse import mybir

    for f in nc.m.functions:
        for bb in f.blocks:
            out = []
            for inst in bb.instructions:
                si = inst.sync_info
                if si is not None and si.on_wait and len(si.on_wait) > max_waits:
                    waits = list(si.on_wait)
                    extra, keep = waits[:-max_waits], waits[-max_waits:]
                    for i in range(0, len(extra), max_waits):
                        nop = mybir.InstNoOp(
                            name=f"I-splitwait-{nc.next_id()}",
                            engine=inst.engine,
                            sync_info=bass_rust.SyncInfo(
                                on_wait=extra[i : i + max_waits], on_update=[]
                            ),
                        )
                        nc.register_instruction(nop)
                        out.append(nop)
                    si.on_wait = keep
                out.append(inst)
            bb.instructions[:] = out


# --------------------------------------------------------------- host prep
def _prep_weights(p):
    g1, b1 = np.asarray(p["ln1_g"], _f32), np.asarray(p["ln1_b"], _f32)
    g2, b2 = np.asarray(p["ln2_g"], _f32), np.asarray(p["ln2_b"], _f32)
    alpha = _f32(1.0 / (1.0 + np.exp(-np.float64(np.asarray(p["mix_weight"])))))

    def fold_ln(W, b, g, bln):
        W = np.asarray(W, _f32)
        return W * g[None, :], W @ bln + np.asarray(b, _f32)

    def aug_T(Wf, bf):
        u = Wf.sum(axis=1)
        return np.concatenate([Wf.T, u[None, :], bf[None, :]], axis=0).astype(_f32)

    def qk_combined(name, scale):
        Wa, ba = fold_ln(p[f"add_{name}_w"], p[f"add_{name}_b"], g1, b1)
        Wm, bm = fold_ln(p[f"mul_{name}_w"], p[f"mul_{name}_b"], g1, b1)
        Wa, ba, Wm, bm = Wa * scale, ba * scale, Wm * scale, bm * scale
        W = np.zeros((2 * D, D), _f32)
        b = np.zeros((2 * D,), _f32)
        for h in range(H):
            W[h * 128 : h * 128 + 64] = Wa[h * 64 : (h + 1) * 64]
            W[h * 128 + 64 : (h + 1) * 128] = Wm[h * 64 : (h + 1) * 64]
            b[h * 128 : h * 128 + 64] = ba[h * 64 : (h + 1) * 64]
            b[h * 128 + 64 : (h + 1) * 128] = bm[h * 64 : (h + 1) * 64]
        return aug_T(W, b)  # [770, 1536]

    Wq = qk_combined("q", _f32(1.0 / SCALE))
    Wk = qk_combined("k", _f32(1.0))

    def v_aug(prefix):
        Wf, bf = fold_ln(p[f"{prefix}_v_w"], p[f"{prefix}_v_b"], g1, b1)
        W = np.zeros((VW, D), _f32)
        b = np.zeros((VW,), _f32)
        for h in range(H):
            W[h * DHA : h * DHA + DH] = Wf[h * DH : (h + 1) * DH]
            b[h * DHA : h * DHA + DH] = bf[h * DH : (h + 1) * DH]
            b[h * DHA + DH] = 1.0  # ones column via bias row
        return W, b

    Wva, bva = v_aug("add")
    Wvm, bvm = v_aug("mul")
    Wv = aug_T(np.concatenate([Wva, Wvm], 0), np.concatenate([bva, bvm], 0))

    Woa = alpha * np.asarray(p["add_o_w"], _f32)
    boa = alpha * np.asarray(p["add_o_b"], _f32)
    Wom = (1.0 - alpha) * np.asarray(p["mul_o_w"], _f32)
    bom = (1.0 - alpha) * np.asarray(p["mul_o_b"], _f32)
    Wo = np.concatenate([Woa.T, Wom.T, (boa + bom)[None, :]], 0).astype(_f32)

    W1 = aug_T(*fold_ln(p["ffn1_w"], p["ffn1_b"], g2, b2))
    W2 = np.concatenate(
        [np.asarray(p["ffn2_w"], _f32).T, np.asarray(p["ffn2_b"], _f32)[None, :]], 0
    )

    return {
        "wq": _f32r(Wq), "wk": _f32r(Wk), "wv": _f32r(Wv),
        "wo": _f32r(Wo), "w1": _f32r(W1), "w2": _f32r(W2),
    }


# ------------------------------------------------------------ device build
_BUILD_CACHE = {}


def _build():
    if "nc" in _BUILD_CACHE:
        return _BUILD_CACHE["nc"]
    import concourse.bass as bass
    import concourse.tile as tile
    from concourse import mybir
    from concourse.masks import make_identity

    dt = mybir.dt
    F32, F32R = dt.float32, dt.float32r
    AF = mybir.ActivationFunctionType
    ALU = mybir.AluOpType

    nc = bass.Bass("TRN2", target_bir_lowering=False, num_devices=N_CORES)

    def act_recip(out_ap, in_ap):
        """ACT Reciprocal (spline) — ~1e-5 rel err, 1 op.  Built directly:
        the bass wrapper refuses Reciprocal for precision reasons that do
        not matter at our tolerance."""
        ins = [
            nc.scalar.lower_ap(in_ap),
            mybir.ImmediateValue(dtype=F32, value=0.0),
            mybir.ImmediateValue(dtype=F32, value=1.0),
            mybir.ImmediateValue(dtype=F32, value=0.0),
        ]
        nc.scalar.add_instruction(
            mybir.InstActivation(
                name=f"I-{nc.next_id()}",
                func=AF.Reciprocal,
                ins=ins,
                outs=[nc.scalar.lower_ap(out_ap)],
            )
        )

    x_ext = nc.declare_dram_parameter("x", [TOK, D], F32, isOutput=False)
    wq_ext = nc.declare_dram_parameter("wq", [D + 2, 2 * D], F32R, isOutput=False)
    wk_ext = nc.declare_dram_parameter("wk", [D + 2, 2 * D], F32R, isOutput=False)
    wv_ext = nc.declare_dram_parameter("wv", [D + 2, 2 * VW], F32R, isOutput=False)
    wo_ext = nc.declare_dram_parameter("wo", [2 * D + 1, D], F32R, isOutput=False)
    w1_ext = nc.declare_dram_parameter("w1", [D + 2, DFF], F32R, isOutput=False)
    w2_ext = nc.declare_dram_parameter("w2", [DFF + 1, D], F32R, isOutput=False)
    out_ext = nc.declare_dram_parameter("out", [TOK, D], F32, isOutput=True)

    x_tiled = x_ext.ap().rearrange("(n p) d -> p n d", p=128)
    out_tiled = out_ext.ap().rearrange("(n p) d -> p n d", p=128)

    with contextlib.ExitStack() as top:
        tc = top.enter_context(tile.TileContext(nc))

        const_pool = top.enter_context(tc.tile_pool(name="const", bufs=1))
        persist = top.enter_context(tc.tile_pool(name="persist", bufs=1))
        arena = top.enter_context(tc.tile_pool(name="arena", bufs=1))
        lnrows = top.enter_context(tc.tile_pool(name="lnrows", bufs=1))
        wpool = top.enter_context(tc.tile_pool(name="wts", bufs=5))

        ident = const_pool.tile([128, 128], F32)
        make_identity(nc, ident[:])
        # f32r constants: memset fp32 staging, ACT-copy to f32r (direct
        # f32r memset fails walrus ISA validation)
        stage = const_pool.tile([128, TOK], F32, name="stage")
        nc.vector.memset(stage[:], 1.0)
        ones_col = const_pool.tile([128, 1], F32R)
        nc.scalar.copy(ones_col[:], stage[:, 0:1])
        ones_row = const_pool.tile([1, TOK], F32R)
        nc.scalar.copy(ones_row[:], stage[0:1, :])
        eps_row = const_pool.tile([1, DH], F32R)
        nc.scalar.activation(eps_row[:], stage[0:1, 0:DH], AF.Copy, scale=EPS)
        eps_col = const_pool.tile([128, 1], F32)
        nc.vector.memset(eps_col[:], EPS)
        lneps_col = const_pool.tile([1, 1], F32)
        nc.vector.memset(lneps_col[:], LN_EPS)

        # persistent feature-major tensor (f32r) with LN aug rows in chunk
        # KC; x2 = x + attention overwrites it in place (disjoint per-batch
        # column slices)
        xaug = persist.tile([128, KC + 1, TOK], F32R, tag="xaug")
        x2aug = xaug
        rstd1_bc = persist.tile([128, TOK], F32, tag="rstd1_bc")
        rstd2_bc = persist.tile([128, TOK], F32, tag="rstd2_bc")
        rstd1_cols = lnrows.tile([128, NTC], F32, tag="rstd1_cols")
        rstd2_cols = lnrows.tile([128, NTC], F32, tag="rstd2_cols")

        def transpose_fm(dst, src_view):
            """token-major [128, NTC, 768] -> feature-major dst chunks."""
            with tc.tile_pool(name="trp", bufs=8, space="PSUM") as trp:
                for c in range(KC):
                    for t in range(NTC):
                        pt = trp.tile([128, 128], F32, tag="tr")
                        nc.tensor.transpose(
                            pt[:], src_view[:, t, c * 128 : (c + 1) * 128], ident[:]
                        )
                        nc.scalar.copy(dst[:, c, t * 128 : (t + 1) * 128], pt[:])

        def ln_stats(src, rstd_bc, rstd_cols, sq, w0, nw):
            """LN stats over token window [w0, w0+nw): write -mu / 1/rstd
            into src chunk-KC rows 0/1 (window slice), fill rstd_bc window
            and per-128-chunk rstd columns."""
            wsl = slice(w0, w0 + nw)
            with (
                tc.tile_pool(name="lnsb", bufs=1) as lnsb,
                tc.tile_pool(name="lnr", bufs=2) as lnr,
                tc.tile_pool(name="lnps", bufs=1, space="PSUM") as lnps,
            ):
                for c in range(KC):
                    nc.scalar.activation(sq[:, c, 0:nw], src[:, c, wsl], AF.Square)
                sum_x = lnsb.tile([1, TOK], F32, tag="sum_x")
                sum_q = lnsb.tile([1, TOK], F32, tag="sum_q")
                for half in range(nw // 512):
                    hs = slice(half * 512, (half + 1) * 512)
                    gs = slice(w0 + half * 512, w0 + (half + 1) * 512)
                    psx = lnps.tile([1, 512], F32, tag="stx")
                    psq = lnps.tile([1, 512], F32, tag="stq")
                    for c in range(KC):
                        nc.tensor.matmul(
                            psx[:], ones_col[:], src[:, c, gs],
                            start=(c == 0), stop=(c == KC - 1),
                        )
                    for c in range(KC):
                        nc.tensor.matmul(
                            psq[:], ones_col[:], sq[:, c, hs],
                            start=(c == 0), stop=(c == KC - 1),
                        )
                    nc.vector.tensor_copy(sum_x[:, hs], psx[:])
                    nc.vector.tensor_copy(sum_q[:, hs], psq[:])
                nc.scalar.activation(
                    src[0:1, KC, wsl], sum_x[:, 0:nw], AF.Copy, scale=-1.0 / D
                )
                mu2 = lnr.tile([1, TOK], F32, tag="mu2")
                nc.scalar.activation(
                    mu2[:, 0:nw], sum_x[:, 0:nw], AF.Square, scale=1.0 / D
                )
                var = lnr.tile([1, TOK], F32, tag="var")
                nc.vector.tensor_scalar(
                    out=var[:, 0:nw], in0=sum_q[:, 0:nw], scalar1=1.0 / D,
                    scalar2=None, op0=ALU.mult,
                )
                nc.vector.tensor_sub(var[:, 0:nw], var[:, 0:nw], mu2[:, 0:nw])
                lnv = lnr.tile([1, TOK], F32, tag="lnv")
                nc.scalar.activation(
                    lnv[:, 0:nw], var[:, 0:nw], AF.Ln, bias=lneps_col[:]
                )
                rstd_row = lnr.tile([1, TOK], F32R, tag="rstd_row")
                nc.scalar.activation(
                    rstd_row[:, 0:nw], lnv[:, 0:nw], AF.Exp, scale=-0.5
                )
                rstd_inv = lnr.tile([1, TOK], F32R, tag="rstd_inv")
                nc.scalar.activation(
                    rstd_inv[:, 0:nw], lnv[:, 0:nw], AF.Exp, scale=0.5
                )
                # partition-1 writes need the DMA path (compute APs must be
                # 32-aligned in partition base)
                nc.sync.dma_start(out=src[1:2, KC, wsl], in_=rstd_inv[:, 0:nw])
                for half in range(nw // 512):
                    hs = slice(half * 512, (half + 1) * 512)
                    gs = slice(w0 + half * 512, w0 + (half + 1) * 512)
                    pb = lnps.tile([128, 512], F32, tag="rb")
                    nc.tensor.matmul(
                        pb[:], ones_row[0:1, 0:128], rstd_row[:, hs],
                        start=True, stop=True,
                    )
                    nc.scalar.copy(rstd_bc[:, gs], pb[:])
                # row -> per-chunk columns via tiny SBUF DMAs
                for t in range(nw // 128):
                    nc.sync.dma_start(
                        out=rstd_cols[:, w0 // 128 + t : w0 // 128 + t + 1],
                        in_=rstd_row[0:1, t * 128 : (t + 1) * 128].bitcast(F32),
                    )

        # ---------------- phase 0: load x, transpose, LN1 stats
        x_tm = arena.tile([128, NTC, D], F32, tag="bigA", name="x_tm")
        for t in range(NTC):
            nc.sync.dma_start(out=x_tm[:, t, :], in_=x_tiled[:, t, :])
        transpose_fm(xaug, x_tm)
        sq1 = arena.tile([128, KC, TOK], F32R, tag="bigB", name="sq1")
        ln_stats(xaug, rstd1_bc, rstd1_cols, sq1, 0, TOK)

        def load_w(wext, c, cols, kpart=128, tag="wt"):
            wt = wpool.tile([128, 512], F32R, tag=tag, name=f"w_{tag}")
            nc.sync.dma_start(
                out=wt[0:kpart, 0 : cols.stop - cols.start],
                in_=wext.ap()[c * 128 : c * 128 + kpart, cols],
            )
            return wt

        # ============ per-batch: QKV -> attention -> O-proj -> FFN ========
        for b in range(BPC):
            tsl = slice(b * 512, (b + 1) * 512)

            with contextlib.ExitStack() as bctx:
                bpool = bctx.enter_context(tc.tile_pool(name=f"bt{b}", bufs=1))
                q_sb = arena.tile([128, FC, 512], F32R, tag="bigA",
                                  name=f"q_sb{b}")
                k_sb = arena.tile([128, FC, 512], F32R, tag="bigB",
                                  name=f"k_sb{b}")

                # ---- q/k projections (feature-major, oc groups of 4)
                with tc.tile_pool(name="pp", bufs=6, space="PSUM") as ppq:
                    for wext, dst in ((wq_ext, q_sb), (wk_ext, k_sb)):
                        for og in range(FC // 4):
                            pss = [
                                ppq.tile([128, 512], F32, tag="pp",
                                         name=f"pp{_j}")
                                for _j in range(4)
                            ]
                            for c in range(KC + 1):
                                kpart = 128 if c < KC else 2
                                wt = load_w(
                                    wext, c,
                                    slice(og * 512, (og + 1) * 512), kpart,
                                )
                                for j in range(4):
                                    nc.tensor.matmul(
                                        pss[j][:],
                                        wt[0:kpart, j * 128 : (j + 1) * 128],
                                        xaug[0:kpart, min(c, KC), tsl],
                                        start=(c == 0), stop=(c == KC),
                                    )
                            for j in range(4):
                                nc.vector.tensor_mul(
                                    dst[:, og * 4 + j, :], pss[j][:],
                                    rstd1_bc[:, tsl],
                                )

                    # ---- v projections (token-major)
                    v_add = bpool.tile([128, 4, VW], F32R, tag="v_add")
                    v_mul = bpool.tile([128, 4, VW], F32R, tag="v_mul")
                    v_sgn = bpool.tile([128, 4, VW], F32R, tag="v_sgn")
                    vdo = [(0, 512), (512, VW - 512), (VW, 512),
                           (VW + 512, VW - 512)]
                    with tc.tile_pool(name="wvp", bufs=7) as wvp:
                        for o0, ow in vdo:
                            wts = []
                            for c in range(KC + 1):
                                kpart = 128 if c < KC else 2
                                wt = wvp.tile([128, 512], F32R, tag="wv",
                                              name="wv_t")
                                nc.sync.dma_start(
                                    out=wt[0:kpart, 0:ow],
                                    in_=wv_ext.ap()[c * 128 : c * 128 + kpart,
                                                    o0 : o0 + ow],
                                )
                                wts.append(wt)
                            for t in range(4):
                                gt = b * 4 + t
                                ps = ppq.tile([128, 512], F32, tag="pp",
                                              name="pv")
                                for c in range(KC + 1):
                                    kpart = 128 if c < KC else 2
                                    nc.tensor.matmul(
                                        ps[:, 0:ow],
                                        xaug[0:kpart, min(c, KC),
                                             gt * 128 : (gt + 1) * 128],
                                        wts[c][0:kpart, 0:ow],
                                        start=(c == 0), stop=(c == KC),
                                    )
                                dst = v_add if o0 < VW else v_mul
                                d0 = o0 if o0 < VW else o0 - VW
                                nc.scalar.activation(
                                    dst[:, t, d0 : d0 + ow], ps[:, 0:ow],
                                    AF.Copy,
                                    scale=rstd1_cols[:, gt : gt + 1],
                                )

                # sign / log(|.|+eps) on the 64-col head blocks of v_mul
                for t in range(4):
                    vm = v_mul[:, t, :].rearrange("p (h w) -> p h w", h=H)[
                        :, :, 0:DH]
                    vs = v_sgn[:, t, :].rearrange("p (h w) -> p h w", h=H)[
                        :, :, 0:DH]
                    nc.scalar.activation(vs, vm, AF.Sign)
                    nc.scalar.activation(vm, vm, AF.Abs)
                    nc.scalar.activation(vm, vm, AF.Ln, bias=eps_col[:])

                # ---- attention (1-head-deep software pipeline)
                av_stack = bpool.tile([128, FC, 512], F32R, tag="av_stack")
                with (
                    tc.tile_pool(name="att", bufs=4) as att,
                    tc.tile_pool(name="att1", bufs=1) as att1,
                    tc.tile_pool(name="attr", bufs=1) as attr,
                    tc.tile_pool(name="aps", bufs=2, space="PSUM") as aps,
                    tc.tile_pool(name="avps", bufs=1, space="PSUM") as avps,
                    tc.tile_pool(name="rbps", bufs=1, space="PSUM") as rbps,
                ):
                    e_pairs = {}

                    def scores_exp(h):
                        es = []
                        for kc4 in range(4):
                            ks = slice(kc4 * 128, (kc4 + 1) * 128)
                            psp = aps.tile([128, 1024], F32, tag="ss",
                                           name=f"ss{h}_{kc4}")
                            nc.tensor.matmul(
                                psp[:, 0:512], k_sb[0:64, h, ks],
                                q_sb[0:64, h, :],
                                start=True, stop=True, tile_position=(0, 0),
                            )
                            nc.tensor.matmul(
                                psp[:, 512:1024], k_sb[64:128, h, ks],
                                q_sb[64:128, h, :],
                                start=True, stop=True, tile_position=(64, 0),
                            )
                            e = att.tile([128, 1024], F32R, tag="e_pair",
                                         name=f"e{h}_{kc4}")
                            nc.scalar.activation(e[:], psp[:], AF.Exp)
                            es.append(e)
                        e_pairs[h] = es

                    def av_epilogue(h):
                        es = e_pairs.pop(h)
                        pa = avps.tile([DHA, 512], F32, tag="pa", name=f"pa{h}")
                        pl = avps.tile([DHA, 512], F32, tag="pl", name=f"pl{h}")
                        pg = avps.tile([DH, 512], F32, tag="pg", name=f"pg{h}")
                        for kc4 in range(4):
                            ea = es[kc4][:, 0:512]
                            em = es[kc4][:, 512:1024]
                            nc.tensor.matmul(
                                pa[:], v_add[:, kc4, h * DHA : h * DHA + DHA],
                                ea, start=(kc4 == 0), stop=(kc4 == 3),
                            )
                            nc.tensor.matmul(
                                pl[:], v_mul[:, kc4, h * DHA : h * DHA + DHA],
                                em, start=(kc4 == 0), stop=(kc4 == 3),
                            )
                            nc.tensor.matmul(
                                pg[:], v_sgn[:, kc4, h * DHA : h * DHA + DH],
                                em, start=(kc4 == 0), stop=False,
                            )
                        # evacuate immediately: the epilogue chain below then
                        # reads SBUF, releasing pa/pl for the next head
                        av_u = att.tile([DHA, 1024], F32, tag="av_u",
                                        name=f"avu{h}", bufs=2)
                        nc.vector.tensor_copy(av_u[:, 0:512], pa[:])
                        nc.vector.tensor_copy(av_u[:, 512:1024], pl[:])
                        s_mul = attr.tile([1, 512], F32R, tag="s_mul",
                                          name=f"sm{h}")
                        nc.vector.tensor_copy(
                            s_mul[:], av_u[DH : DH + 1, 512:1024]
                        )
                        nc.tensor.matmul(
                            pg[:], eps_row[:], s_mul[:], start=False, stop=True
                        )
                        sgn = att1.tile([DH, 512], F32R, tag="sgn")
                        nc.scalar.activation(sgn[:], pg[:], AF.Sign)
                        # add-branch normalize: 1/S = exp(-ln(S))
                        lna = attr.tile([1, 512], F32, tag="lna",
                                        name=f"lna{h}")
                        nc.scalar.activation(
                            lna[:], av_u[DH : DH + 1, 0:512], AF.Ln
                        )
                        raf = attr.tile([1, 512], F32R, tag="raf",
                                        name=f"raf{h}")
                        nc.scalar.activation(raf[:], lna[:], AF.Exp, scale=-1.0)
                        prb = rbps.tile([DH, 512], F32, tag="prb",
                                        name=f"prb{h}")
                        nc.tensor.matmul(
                            prb[:], ones_row[0:1, 0:DH], raf[:],
                            start=True, stop=True,
                        )
                        rab = att1.tile([DH, 512], F32, tag="rab")
                        nc.vector.tensor_copy(rab[:], prb[:])
                        nc.vector.tensor_mul(
                            av_stack[(h % 2) * 64 : (h % 2) * 64 + 64,
                                     h // 2, :],
                            av_u[0:DH, 0:512], rab[:],
                        )
                        # mul-branch normalize + sign
                        lnm = attr.tile([1, 512], F32, tag="lnm",
                                        name=f"lnm{h}")
                        nc.scalar.activation(
                            lnm[:], av_u[DH : DH + 1, 512:1024], AF.Ln
                        )
                        rmf = attr.tile([1, 512], F32R, tag="rmf",
                                        name=f"rmf{h}")
                        nc.scalar.activation(rmf[:], lnm[:], AF.Exp, scale=-1.0)
                        pmb = rbps.tile([DH, 512], F32, tag="prb",
                                        name=f"pmb{h}")
                        nc.tensor.matmul(
                            pmb[:], ones_row[0:1, 0:DH], rmf[:],
                            start=True, stop=True,
                        )
                        rmb = att1.tile([DH, 512], F32, tag="rmb")
                        nc.vector.tensor_copy(rmb[:], pmb[:])
                        lon = att1.tile([DH, 512], F32, tag="lon")
                        nc.vector.tensor_mul(lon[:], av_u[0:DH, 512:1024],
                                             rmb[:])
                        elo = att1.tile([DH, 512], F32R, tag="elo")
                        nc.scalar.activation(elo[:], lon[:], AF.Exp)
                        nc.gpsimd.tensor_mul(
                            av_stack[(h % 2) * 64 : (h % 2) * 64 + 64,
                                     6 + h // 2, :],
                            elo[:], sgn[:],
                        )

                    for step in range(H + 1):
                        if step < H:
                            scores_exp(step)
                        if step >= 1:
                            av_epilogue(step - 1)

                # ---- O-projection + mix + residual -> x2 (feature-major)
                with tc.tile_pool(name="ops", bufs=6, space="PSUM") as ops:
                    for og in range(2):
                        n_out = 4 if og == 0 else 2
                        pss = [
                            ops.tile([128, 512], F32, tag="po",
                                     name=f"po{_j}")
                            for _j in range(n_out)
                        ]
                        for c in range(FC + 1):
                            kpart = 128 if c < FC else 1
                            wt = load_w(
                                wo_ext, c,
                                slice(og * 512, og * 512 + n_out * 128), kpart,
                            )
                            for j in range(n_out):
                                src = (
                                    av_stack[:, c, :] if c < FC
                                    else ones_row[:, 0:512]
                                )
                                nc.tensor.matmul(
                                    pss[j][0:128, :],
                                    wt[0:kpart, j * 128 : (j + 1) * 128],
                                    src[0:kpart, :] if c < FC else src,
                                    start=(c == 0), stop=(c == FC),
                                )
                        for j in range(n_out):
                            oc = og * 4 + j
                            nc.vector.tensor_add(
                                x2aug[:, oc, tsl], pss[j][:], xaug[:, oc, tsl]
                            )


        # ================ LN2 stats (both batches) + FFN per half =========
        sq2 = arena.tile([128, KC, TOK], F32R, tag="bigB", name="sq2")
        ln_stats(x2aug, rstd2_bc, rstd2_cols, sq2, 0, TOK)

        with (
            tc.tile_pool(name="gsb", bufs=1) as gsb,
            tc.tile_pool(name="fwork", bufs=3) as fwork,
            tc.tile_pool(name="orow", bufs=2) as orow,
            tc.tile_pool(name="fps", bufs=6, space="PSUM") as fps,
            tc.tile_pool(name="trp2", bufs=2, space="PSUM") as trp2,
        ):
            for b in range(BPC):
                hsl = slice(b * 512, (b + 1) * 512)
                g_sb = gsb.tile([128, FFC, 512], F32R, tag="g_sb",
                                name=f"g_sb{b}")
                for og in range(FFC // 4):
                    pss = [fps.tile([128, 512], F32, tag="pf", name=f"pf{_j}")
                           for _j in range(4)]
                    for c in range(KC + 1):
                        kpart = 128 if c < KC else 2
                        wt = load_w(
                            w1_ext, c, slice(og * 512, (og + 1) * 512), kpart,
                        )
                        for j in range(4):
                            nc.tensor.matmul(
                                pss[j][:],
                                wt[0:kpart, j * 128 : (j + 1) * 128],
                                x2aug[0:kpart, min(c, KC), hsl],
                                start=(c == 0), stop=(c == KC),
                            )
                    for j in range(4):
                        pre = fwork.tile([128, 512], F32, tag="pre")
                        nc.vector.tensor_mul(pre[:], pss[j][:], rstd2_bc[:, hsl])
                        nc.scalar.activation(
                            g_sb[:, og * 4 + j, :], pre[:], AF.Gelu
                        )

                # FFN2 + residual -> out (feature-major, fp32)
                out_fm = gsb.tile([128, KC, 512], F32, tag="out_fm",
                                  name=f"out_fm{b}")
                for og in range(2):
                    n_out = 4 if og == 0 else 2
                    pss = [
                        fps.tile([128, 512], F32, tag="pf", name=f"pf2{_j}")
                        for _j in range(n_out)
                    ]
                    for c in range(FFC + 1):
                        kpart = 128 if c < FFC else 1
                        wt = load_w(
                            w2_ext, c,
                            slice(og * 512, og * 512 + n_out * 128), kpart,
                        )
                        for j in range(n_out):
                            src = g_sb[:, c, :] if c < FFC else ones_row[:, hsl]
                            nc.tensor.matmul(
                                pss[j][:],
                                wt[0:kpart, j * 128 : (j + 1) * 128],
                                src,
                                start=(c == 0), stop=(c == FFC),
                            )
                    for j in range(n_out):
                        oc = og * 4 + j
                        nc.vector.tensor_add(
                            out_fm[:, oc, :], pss[j][:], x2aug[:, oc, hsl]
                        )

                # transpose to token-major, one DMA per 128-token row block
                for t in range(4):
                    row = orow.tile([128, D], F32, tag="row")
                    for c in range(KC):
                        pt = trp2.tile([128, 128], F32, tag="tr2")
                        nc.tensor.transpose(
                            pt[:], out_fm[:, c, t * 128 : (t + 1) * 128],
                            ident[:],
                        )
                        nc.scalar.copy(row[:, c * 128 : (c + 1) * 128], pt[:])
                    nc.sync.dma_start(
                        out=out_tiled[:, b * 4 + t, :], in_=row[:]
                    )

    _split_sync_waits(nc, max_waits=1)
    _BUILD_CACHE["nc"] = nc
    return nc


# ---------------------------------------------------------------- kernel()
LAST_EXEC_TIME_NS = None


def kernel(x, params):
    global LAST_EXEC_TIME_NS
    _install_shims()
    from concourse.bass_utils import run_bass_kernel_spmd

    x = np.ascontiguousarray(np.asarray(x, _f32))
    prep = _prep_weights(params)
    nc = _build()

    in_maps = []
    for c in range(N_CORES):
        shard = x[c * BPC : (c + 1) * BPC].reshape(TOK, D)
        in_maps.append({"x": np.ascontiguousarray(shard), **prep})

    trace = bool(int(os.environ.get("KBENCH_TRACE", "0")))
    res = run_bass_kernel_spmd(nc, in_maps, list(range(N_CORES)), trace=trace)
    LAST_EXEC_TIME_NS = res.exec_time_ns

    out = np.empty((B, S, D), _f32)
    for c in range(N_CORES):
        out[c * BPC : (c + 1) * BPC] = res.results[c]["out"].reshape(BPC, S, D)
    return out


# revision 28
# speedup vs baseline: 1.0959x; 1.0601x over previous
"""AMFormer layer on 8 Trainium2 NeuronCores.

Sharding: data-parallel over batch (16 batches -> 2 per core), zero
collectives.  All matmuls run as float32r (11-bit mantissa, fp32
accumulate, 1 PE cycle/row).  LayerNorms are folded into the following
projections via two augmented contraction rows (-mu, 1/rstd) whose
weight-side rows (column sums u, folded bias b') are precomputed on the
host.  add/mul attention heads are interleaved in the combined Q/K
weights; each k-chunk's two score matmuls write the two halves of one
2-bank PSUM tile so a single ACT exp evacuates both.  V projections are
emitted token-major in a 65-column-per-head layout whose 65th column is
a bias-produced constant 1.0, so the attention-value matmuls also
produce softmax denominators for free.  The multiplicative branch's
sign() threshold uses sign(sign_w + EPS*sumexp) == sign(sign_w/sumexp +
EPS), accumulated into the same PSUM tile with one rank-1 matmul.
Attention is software-pipelined one head deep (scores/exp of head h+1
issue before the AV matmuls of head h) to keep the in-order PE stream
from stalling on ACT.  LN2 + FFN run per 512-token batch so their dense
matmuls can overlap the other batch's attention phase.
"""

import os
import sys
import types
import contextlib
import numpy as np

# ---------------------------------------------------------------- constants
B, S, D = 16, 512, 768
H, DH = 12, 64
DFF = 4 * D
N_CORES = 8
BPC = B // N_CORES          # batches per core
TOK = BPC * S               # tokens per core (1024)
NTC = TOK // 128            # token chunks per core (8)
SCALE = float(np.sqrt(DH))
EPS = 1e-6
LN_EPS = 1e-5
DHA = DH + 1                # augmented head width in V layout (65)
VW = H * DHA                # augmented V width per branch (780)
KC = D // 128               # feature chunks (6)
FC = 2 * D // 128           # combined q/k output chunks (12)
FFC = DFF // 128            # ffn hidden chunks (24)

_f32 = np.float32


def _f32r(a):
    """Round fp32 -> float32r bit pattern (11 mantissa bits, round-nearest)."""
    a = np.ascontiguousarray(a, _f32)
    u = a.view(np.uint32).copy()
    u = (u + np.uint32(0x800)) & np.uint32(0xFFFFF000)
    return u.view(np.float32)


# ------------------------------------------------------------- axon shims
def _install_shims():
    if "antenv.axon_hooks" not in sys.modules:
        try:
            import trn_agent_boot.trn_boot as tb
            hook = tb._ntff_profile_via_ctypes("/opt/axon/libaxon_pjrt.so")
        except Exception:
            hook = None
        mod = types.ModuleType("antenv.axon_hooks")
        mod.get_axon_ntff_profile_hook = lambda: hook
        mod.set_axon_ntff_profile_hook = lambda h: None
        sys.modules["antenv.axon_hooks"] = mod
    try:
        import concourse.bass_utils as bu
        bu.upload_artifacts = lambda tmpdir: f"local:{tmpdir}"
    except Exception:
        pass


# ------------------------------------------------- walrus sync-wait limiter
def _split_sync_waits(nc, max_waits=1):
    """This container's walrus accepts a single sync-wait per instruction;
    move extras onto same-engine NOPs placed immediately before."""
    import bass_rust
    from concourse import mybir

    for f in nc.m.functions:
        for bb in f.blocks:
            out = []
            for inst in bb.instructions:
                si = inst.sync_info
                if si is not None and si.on_wait and len(si.on_wait) > max_waits:
                    waits = list(si.on_wait)
                    extra, keep = waits[:-max_waits], waits[-max_waits:]
                    for i in range(0, len(extra), max_waits):
                        nop = mybir.InstNoOp(
                            name=f"I-splitwait-{nc.next_id()}",
                            engine=inst.engine,
                            sync_info=bass_rust.SyncInfo(
                                on_wait=extra[i : i + max_waits], on_update=[]
                            ),
                        )
                        nc.register_instruction(nop)
                        out.append(nop)
                    si.on_wait = keep
                out.append(inst)
            bb.instructions[:] = out


# --------------------------------------------------------------- host prep
def _prep_weights(p):
    g1, b1 = np.asarray(p["ln1_g"], _f32), np.asarray(p["ln1_b"], _f32)
    g2, b2 = np.asarray(p["ln2_g"], _f32), np.asarray(p["ln2_b"], _f32)
    alpha = _f32(1.0 / (1.0 + np.exp(-np.float64(np.asarray(p["mix_weight"])))))

    def fold_ln(W, b, g, bln):
        W = np.asarray(W, _f32)
        return W * g[None, :], W @ bln + np.asarray(b, _f32)

    def aug_T(Wf, bf):
        u = Wf.sum(axis=1)
        return np.concatenate([Wf.T, u[None, :], bf[None, :]], axis=0).astype(_f32)

    def qk_combined(name, scale):
        Wa, ba = fold_ln(p[f"add_{name}_w"], p[f"add_{name}_b"], g1, b1)
        Wm, bm = fold_ln(p[f"mul_{name}_w"], p[f"mul_{name}_b"], g1, b1)
        Wa, ba, Wm, bm = Wa * scale, ba * scale, Wm * scale, bm * scale
        W = np.zeros((2 * D, D), _f32)
        b = np.zeros((2 * D,), _f32)
        for h in range(H):
            W[h * 128 : h * 128 + 64] = Wa[h * 64 : (h + 1) * 64]
            W[h * 128 + 64 : (h + 1) * 128] = Wm[h * 64 : (h + 1) * 64]
            b[h * 128 : h * 128 + 64] = ba[h * 64 : (h + 1) * 64]
            b[h * 128 + 64 : (h + 1) * 128] = bm[h * 64 : (h + 1) * 64]
        return aug_T(W, b)  # [770, 1536]

    Wq = qk_combined("q", _f32(1.0 / SCALE))
    Wk = qk_combined("k", _f32(1.0))

    def v_aug(prefix):
        Wf, bf = fold_ln(p[f"{prefix}_v_w"], p[f"{prefix}_v_b"], g1, b1)
        W = np.zeros((VW, D), _f32)
        b = np.zeros((VW,), _f32)
        for h in range(H):
            W[h * DHA : h * DHA + DH] = Wf[h * DH : (h + 1) * DH]
            b[h * DHA : h * DHA + DH] = bf[h * DH : (h + 1) * DH]
            b[h * DHA + DH] = 1.0  # ones column via bias row
        return W, b

    Wva, bva = v_aug("add")
    Wvm, bvm = v_aug("mul")
    Wv = aug_T(np.concatenate([Wva, Wvm], 0), np.concatenate([bva, bvm], 0))

    Woa = alpha * np.asarray(p["add_o_w"], _f32)
    boa = alpha * np.asarray(p["add_o_b"], _f32)
    Wom = (1.0 - alpha) * np.asarray(p["mul_o_w"], _f32)
    bom = (1.0 - alpha) * np.asarray(p["mul_o_b"], _f32)
    Wo = np.concatenate([Woa.T, Wom.T, (boa + bom)[None, :]], 0).astype(_f32)

    W1 = aug_T(*fold_ln(p["ffn1_w"], p["ffn1_b"], g2, b2))
    W2 = np.concatenate(
        [np.asarray(p["ffn2_w"], _f32).T, np.asarray(p["ffn2_b"], _f32)[None, :]], 0
    )

    return {
        "wq": _f32r(Wq), "wk": _f32r(Wk), "wv": _f32r(Wv),
        "wo": _f32r(Wo), "w1": _f32r(W1), "w2": _f32r(W2),
    }


# ------------------------------------------------------------ device build
_BUILD_CACHE = {}


def _build():
    if "nc" in _BUILD_CACHE:
        return _BUILD_CACHE["nc"]
    import concourse.bass as bass
    import concourse.tile as tile
    from concourse import mybir
    from concourse.masks import make_identity

    dt = mybir.dt
    F32, F32R = dt.float32, dt.float32r
    AF = mybir.ActivationFunctionType
    ALU = mybir.AluOpType

    nc = bass.Bass("TRN2", target_bir_lowering=False, num_devices=N_CORES)

    def act_recip(out_ap, in_ap):
        """ACT Reciprocal (spline) — ~1e-5 rel err, 1 op.  Built directly:
        the bass wrapper refuses Reciprocal for precision reasons that do
        not matter at our tolerance."""
        ins = [
            nc.scalar.lower_ap(in_ap),
            mybir.ImmediateValue(dtype=F32, value=0.0),
            mybir.ImmediateValue(dtype=F32, value=1.0),
            mybir.ImmediateValue(dtype=F32, value=0.0),
        ]
        nc.scalar.add_instruction(
            mybir.InstActivation(
                name=f"I-{nc.next_id()}",
                func=AF.Reciprocal,
                ins=ins,
                outs=[nc.scalar.lower_ap(out_ap)],
            )
        )

    x_ext = nc.declare_dram_parameter("x", [TOK, D], F32, isOutput=False)
    wq_ext = nc.declare_dram_parameter("wq", [D + 2, 2 * D], F32R, isOutput=False)
    wk_ext = nc.declare_dram_parameter("wk", [D + 2, 2 * D], F32R, isOutput=False)
    wv_ext = nc.declare_dram_parameter("wv", [D + 2, 2 * VW], F32R, isOutput=False)
    wo_ext = nc.declare_dram_parameter("wo", [2 * D + 1, D], F32R, isOutput=False)
    w1_ext = nc.declare_dram_parameter("w1", [D + 2, DFF], F32R, isOutput=False)
    w2_ext = nc.declare_dram_parameter("w2", [DFF + 1, D], F32R, isOutput=False)
    out_ext = nc.declare_dram_parameter("out", [TOK, D], F32, isOutput=True)

    x_tiled = x_ext.ap().rearrange("(n p) d -> p n d", p=128)
    out_tiled = out_ext.ap().rearrange("(n p) d -> p n d", p=128)

    with contextlib.ExitStack() as top:
        tc = top.enter_context(tile.TileContext(nc))

        const_pool = top.enter_context(tc.tile_pool(name="const", bufs=1))
        persist = top.enter_context(tc.tile_pool(name="persist", bufs=1))
        arena = top.enter_context(tc.tile_pool(name="arena", bufs=1))
        lnrows = top.enter_context(tc.tile_pool(name="lnrows", bufs=1))
        wpool = top.enter_context(tc.tile_pool(name="wts", bufs=5))

        ident = const_pool.tile([128, 128], F32)
        make_identity(nc, ident[:])
        # f32r constants: memset fp32 staging, ACT-copy to f32r (direct
        # f32r memset fails walrus ISA validation)
        stage = const_pool.tile([128, 512], F32, name="stage")
        nc.vector.memset(stage[:], 1.0)
        ones_col = const_pool.tile([128, 1], F32R)
        nc.scalar.copy(ones_col[:], stage[:, 0:1])
        ones_row = const_pool.tile([1, TOK], F32R)
        nc.scalar.copy(ones_row[:, 0:512], stage[0:1, :])
        nc.scalar.copy(ones_row[:, 512:1024], stage[0:1, :])
        eps_row = const_pool.tile([1, DH], F32R)
        nc.scalar.activation(eps_row[:], stage[0:1, 0:DH], AF.Copy, scale=EPS)
        eps_col = const_pool.tile([128, 1], F32)
        nc.vector.memset(eps_col[:], EPS)
        lneps_col = const_pool.tile([1, 1], F32)
        nc.vector.memset(lneps_col[:], LN_EPS)

        # persistent feature-major tensor (f32r) with LN aug rows in chunk
        # KC; x2 = x + attention overwrites it in place (disjoint per-batch
        # column slices)
        xaug = persist.tile([128, KC + 1, TOK], F32R, tag="xaug")
        x2aug = xaug
        rstd1_bc = persist.tile([128, TOK], F32, tag="rstd1_bc")
        rstd2_bc = persist.tile([128, TOK], F32, tag="rstd2_bc")
        rstd1_cols = lnrows.tile([128, NTC], F32, tag="rstd1_cols")
        rstd2_cols = lnrows.tile([128, NTC], F32, tag="rstd2_cols")

        def transpose_fm(dst, src_view):
            """token-major [128, NTC, 768] -> feature-major dst chunks."""
            with tc.tile_pool(name="trp", bufs=8, space="PSUM") as trp:
                for c in range(KC):
                    for t in range(NTC):
                        pt = trp.tile([128, 128], F32, tag="tr")
                        nc.tensor.transpose(
                            pt[:], src_view[:, t, c * 128 : (c + 1) * 128], ident[:]
                        )
                        nc.scalar.copy(dst[:, c, t * 128 : (t + 1) * 128], pt[:])

        def ln_stats(src, rstd_bc, rstd_cols, sq, w0, nw):
            """LN stats over token window [w0, w0+nw): write -mu / 1/rstd
            into src chunk-KC rows 0/1 (window slice), fill rstd_bc window
            and per-128-chunk rstd columns."""
            wsl = slice(w0, w0 + nw)
            with (
                tc.tile_pool(name="lnsb", bufs=1) as lnsb,
                tc.tile_pool(name="lnr", bufs=2) as lnr,
                tc.tile_pool(name="lnps", bufs=1, space="PSUM") as lnps,
            ):
                for c in range(KC):
                    nc.scalar.activation(sq[:, c, 0:nw], src[:, c, wsl], AF.Square)
                sum_x = lnsb.tile([1, TOK], F32, tag="sum_x")
                sum_q = lnsb.tile([1, TOK], F32, tag="sum_q")
                for half in range(nw // 512):
                    hs = slice(half * 512, (half + 1) * 512)
                    gs = slice(w0 + half * 512, w0 + (half + 1) * 512)
                    psx = lnps.tile([1, 512], F32, tag="stx")
                    psq = lnps.tile([1, 512], F32, tag="stq")
                    for c in range(KC):
                        nc.tensor.matmul(
                            psx[:], ones_col[:], src[:, c, gs],
                            start=(c == 0), stop=(c == KC - 1),
                        )
                    for c in range(KC):
                        nc.tensor.matmul(
                            psq[:], ones_col[:], sq[:, c, hs],
                            start=(c == 0), stop=(c == KC - 1),
                        )
                    nc.vector.tensor_copy(sum_x[:, hs], psx[:])
                    nc.vector.tensor_copy(sum_q[:, hs], psq[:])
                nc.scalar.activation(
                    src[0:1, KC, wsl], sum_x[:, 0:nw], AF.Copy, scale=-1.0 / D
                )
                mu2 = lnr.tile([1, TOK], F32, tag="mu2")
                nc.scalar.activation(
                    mu2[:, 0:nw], sum_x[:, 0:nw], AF.Square, scale=1.0 / D
                )
                var = lnr.tile([1, TOK], F32, tag="var")
                nc.vector.tensor_scalar(
                    out=var[:, 0:nw], in0=sum_q[:, 0:nw], scalar1=1.0 / D,
                    scalar2=None, op0=ALU.mult,
                )
                nc.vector.tensor_sub(var[:, 0:nw], var[:, 0:nw], mu2[:, 0:nw])
                lnv = lnr.tile([1, TOK], F32, tag="lnv")
                nc.scalar.activation(
                    lnv[:, 0:nw], var[:, 0:nw], AF.Ln, bias=lneps_col[:]
                )
                rstd_row = lnr.tile([1, TOK], F32R, tag="rstd_row")
                nc.scalar.activation(
                    rstd_row[:, 0:nw], lnv[:, 0:nw], AF.Exp, scale=-0.5
                )
                rstd_inv = lnr.tile([1, TOK], F32R, tag="rstd_inv")
                nc.scalar.activation(
                    rstd_inv[:, 0:nw], lnv[:, 0:nw], AF.Exp, scale=0.5
                )
                # partition-1 writes need the DMA path (compute APs must be
                # 32-aligned in partition base)
                nc.sync.dma_start(out=src[1:2, KC, wsl], in_=rstd_inv[:, 0:nw])
                for half in range(nw // 512):
                    hs = slice(half * 512, (half + 1) * 512)
                    gs = slice(w0 + half * 512, w0 + (half + 1) * 512)
                    pb = lnps.tile([128, 512], F32, tag="rb")
                    nc.tensor.matmul(
                        pb[:], ones_row[0:1, 0:128], rstd_row[:, hs],
                        start=True, stop=True,
                    )
                    nc.scalar.copy(rstd_bc[:, gs], pb[:])
                # row -> per-chunk columns via tiny SBUF DMAs
                for t in range(nw // 128):
                    nc.sync.dma_start(
                        out=rstd_cols[:, w0 // 128 + t : w0 // 128 + t + 1],
                        in_=rstd_row[0:1, t * 128 : (t + 1) * 128].bitcast(F32),
                    )

        # ---------------- phase 0: load x, transpose, LN1 stats
        x_tm = arena.tile([128, NTC, D], F32, tag="bigA", name="x_tm")
        for t in range(NTC):
            nc.sync.dma_start(out=x_tm[:, t, :], in_=x_tiled[:, t, :])
        transpose_fm(xaug, x_tm)
        sq1 = arena.tile([128, KC, TOK], F32R, tag="bigB", name="sq1")
        ln_stats(xaug, rstd1_bc, rstd1_cols, sq1, 0, TOK)

        def load_w(wext, c, cols, kpart=128, tag="wt"):
            wt = wpool.tile([128, 512], F32R, tag=tag, name=f"w_{tag}")
            nc.sync.dma_start(
                out=wt[0:kpart, 0 : cols.stop - cols.start],
                in_=wext.ap()[c * 128 : c * 128 + kpart, cols],
            )
            return wt

        # ============ per-batch: QKV -> attention -> O-proj -> FFN ========
        for b in range(BPC):
            tsl = slice(b * 512, (b + 1) * 512)

            with contextlib.ExitStack() as bctx:
                bpool = bctx.enter_context(tc.tile_pool(name=f"bt{b}", bufs=1))
                q_sb = arena.tile([128, FC, 512], F32R, tag="bigA",
                                  name=f"q_sb{b}")
                k_sb = arena.tile([128, FC, 512], F32R, tag="bigB",
                                  name=f"k_sb{b}")

                # ---- q/k projections (feature-major, oc groups of 4)
                with tc.tile_pool(name="pp", bufs=6, space="PSUM") as ppq:
                    for wext, dst in ((wq_ext, q_sb), (wk_ext, k_sb)):
                        for og in range(FC // 4):
                            pss = [
                                ppq.tile([128, 512], F32, tag="pp",
                                         name=f"pp{_j}")
                                for _j in range(4)
                            ]
                            for c in range(KC + 1):
                                kpart = 128 if c < KC else 2
                                wt = load_w(
                                    wext, c,
                                    slice(og * 512, (og + 1) * 512), kpart,
                                )
                                for j in range(4):
                                    nc.tensor.matmul(
                                        pss[j][:],
                                        wt[0:kpart, j * 128 : (j + 1) * 128],
                                        xaug[0:kpart, min(c, KC), tsl],
                                        start=(c == 0), stop=(c == KC),
                                    )
                            for j in range(4):
                                nc.vector.tensor_mul(
                                    dst[:, og * 4 + j, :], pss[j][:],
                                    rstd1_bc[:, tsl],
                                )

                    # ---- v projections (token-major)
                    v_add = bpool.tile([128, 4, VW], F32R, tag="v_add")
                    v_mul = bpool.tile([128, 4, VW], F32R, tag="v_mul")
                    v_sgn = bpool.tile([128, 4, VW], F32R, tag="v_sgn")
                    vdo = [(0, 512), (512, VW - 512), (VW, 512),
                           (VW + 512, VW - 512)]
                    with tc.tile_pool(name="wvp", bufs=7) as wvp:
                        for o0, ow in vdo:
                            wts = []
                            for c in range(KC + 1):
                                kpart = 128 if c < KC else 2
                                wt = wvp.tile([128, 512], F32R, tag="wv",
                                              name="wv_t")
                                nc.sync.dma_start(
                                    out=wt[0:kpart, 0:ow],
                                    in_=wv_ext.ap()[c * 128 : c * 128 + kpart,
                                                    o0 : o0 + ow],
                                )
                                wts.append(wt)
                            for t in range(4):
                                gt = b * 4 + t
                                ps = ppq.tile([128, 512], F32, tag="pp",
                                              name="pv")
                                for c in range(KC + 1):
                                    kpart = 128 if c < KC else 2
                                    nc.tensor.matmul(
                                        ps[:, 0:ow],
                                        xaug[0:kpart, min(c, KC),
                                             gt * 128 : (gt + 1) * 128],
                                        wts[c][0:kpart, 0:ow],
                                        start=(c == 0), stop=(c == KC),
                                    )
                                dst = v_add if o0 < VW else v_mul
                                d0 = o0 if o0 < VW else o0 - VW
                                nc.scalar.activation(
                                    dst[:, t, d0 : d0 + ow], ps[:, 0:ow],
                                    AF.Copy,
                                    scale=rstd1_cols[:, gt : gt + 1],
                                )

                # sign / log(|.|+eps) on the 64-col head blocks of v_mul
                for t in range(4):
                    vm = v_mul[:, t, :].rearrange("p (h w) -> p h w", h=H)[
                        :, :, 0:DH]
                    vs = v_sgn[:, t, :].rearrange("p (h w) -> p h w", h=H)[
                        :, :, 0:DH]
                    nc.scalar.activation(vs, vm, AF.Sign)
                    nc.scalar.activation(vm, vm, AF.Abs)
                    nc.scalar.activation(vm, vm, AF.Ln, bias=eps_col[:])

                # ---- attention (1-head-deep software pipeline)
                av_stack = bpool.tile([128, FC, 512], F32R, tag="av_stack")
                with (
                    tc.tile_pool(name="att", bufs=5) as att,
                    tc.tile_pool(name="att1", bufs=1) as att1,
                    tc.tile_pool(name="attr", bufs=1) as attr,
                    tc.tile_pool(name="aps", bufs=2, space="PSUM") as aps,
                    tc.tile_pool(name="avps", bufs=1, space="PSUM") as avps,
                    tc.tile_pool(name="rbps", bufs=1, space="PSUM") as rbps,
                ):
                    e_pairs = {}

                    def scores_exp(h):
                        es = []
                        for kc4 in range(4):
                            ks = slice(kc4 * 128, (kc4 + 1) * 128)
                            psp = aps.tile([128, 1024], F32, tag="ss",
                                           name=f"ss{h}_{kc4}")
                            nc.tensor.matmul(
                                psp[:, 0:512], k_sb[0:64, h, ks],
                                q_sb[0:64, h, :],
                                start=True, stop=True, tile_position=(0, 0),
                            )
                            nc.tensor.matmul(
                                psp[:, 512:1024], k_sb[64:128, h, ks],
                                q_sb[64:128, h, :],
                                start=True, stop=True, tile_position=(64, 0),
                            )
                            e = att.tile([128, 1024], F32R, tag="e_pair",
                                         name=f"e{h}_{kc4}")
                            nc.scalar.activation(e[:], psp[:], AF.Exp)
                            es.append(e)
                        e_pairs[h] = es

                    def av_epilogue(h):
                        es = e_pairs.pop(h)
                        pa = avps.tile([DHA, 512], F32, tag="pa", name=f"pa{h}")
                        pl = avps.tile([DHA, 512], F32, tag="pl", name=f"pl{h}")
                        pg = avps.tile([DH, 512], F32, tag="pg", name=f"pg{h}")
                        for kc4 in range(4):
                            ea = es[kc4][:, 0:512]
                            em = es[kc4][:, 512:1024]
                            nc.tensor.matmul(
                                pa[:], v_add[:, kc4, h * DHA : h * DHA + DHA],
                                ea, start=(kc4 == 0), stop=(kc4 == 3),
                            )
                            nc.tensor.matmul(
                                pl[:], v_mul[:, kc4, h * DHA : h * DHA + DHA],
                                em, start=(kc4 == 0), stop=(kc4 == 3),
                            )
                            nc.tensor.matmul(
                                pg[:], v_sgn[:, kc4, h * DHA : h * DHA + DH],
                                em, start=(kc4 == 0), stop=False,
                            )
                        # evacuate immediately: the epilogue chain below then
                        # reads SBUF, releasing pa/pl for the next head
                        av_u = att.tile([DHA, 1024], F32, tag="av_u",
                                        name=f"avu{h}", bufs=2)
                        nc.vector.tensor_copy(av_u[:, 0:512], pa[:])
                        nc.vector.tensor_copy(av_u[:, 512:1024], pl[:])
                        s_mul = attr.tile([1, 512], F32R, tag="s_mul",
                                          name=f"sm{h}")
                        nc.vector.tensor_copy(
                            s_mul[:], av_u[DH : DH + 1, 512:1024]
                        )
                        nc.tensor.matmul(
                            pg[:], eps_row[:], s_mul[:], start=False, stop=True
                        )
                        sgn = att1.tile([DH, 512], F32R, tag="sgn")
                        nc.scalar.activation(sgn[:], pg[:], AF.Sign)
                        # add-branch normalize: 1/S = exp(-ln(S))
                        lna = attr.tile([1, 512], F32, tag="lnx",
                                        name=f"lna{h}")
                        nc.scalar.activation(
                            lna[:], av_u[DH : DH + 1, 0:512], AF.Ln
                        )
                        raf = attr.tile([1, 512], F32R, tag="raf",
                                        name=f"raf{h}")
                        nc.scalar.activation(raf[:], lna[:], AF.Exp, scale=-1.0)
                        prb = rbps.tile([DH, 512], F32, tag="prb",
                                        name=f"prb{h}")
                        nc.tensor.matmul(
                            prb[:], ones_row[0:1, 0:DH], raf[:],
                            start=True, stop=True,
                        )
                        rab = att1.tile([DH, 512], F32, tag="rab")
                        nc.vector.tensor_copy(rab[:], prb[:])
                        nc.vector.tensor_mul(
                            av_stack[(h % 2) * 64 : (h % 2) * 64 + 64,
                                     h // 2, :],
                            av_u[0:DH, 0:512], rab[:],
                        )
                        # mul-branch normalize + sign
                        lnm = attr.tile([1, 512], F32, tag="lnx",
                                        name=f"lnm{h}")
                        nc.scalar.activation(
                            lnm[:], av_u[DH : DH + 1, 512:1024], AF.Ln
                        )
                        rmf = attr.tile([1, 512], F32R, tag="rmf",
                                        name=f"rmf{h}")
                        nc.scalar.activation(rmf[:], lnm[:], AF.Exp, scale=-1.0)
                        pmb = rbps.tile([DH, 512], F32, tag="prb",
                                        name=f"pmb{h}")
                        nc.tensor.matmul(
                            pmb[:], ones_row[0:1, 0:DH], rmf[:],
                            start=True, stop=True,
                        )
                        rmb = att1.tile([DH, 512], F32, tag="rmb")
                        nc.vector.tensor_copy(rmb[:], pmb[:])
                        lon = att1.tile([DH, 512], F32, tag="lon")
                        nc.vector.tensor_mul(lon[:], av_u[0:DH, 512:1024],
                                             rmb[:])
                        elo = att1.tile([DH, 512], F32R, tag="elo")
                        nc.scalar.activation(elo[:], lon[:], AF.Exp)
                        nc.gpsimd.tensor_mul(
                            av_stack[(h % 2) * 64 : (h % 2) * 64 + 64,
                                     6 + h // 2, :],
                            elo[:], sgn[:],
                        )

                    for step in range(H + 1):
                        if step < H:
                            scores_exp(step)
                        if step >= 1:
                            av_epilogue(step - 1)

                # ---- O-projection + mix + residual -> x2 (feature-major)
                with tc.tile_pool(name="ops", bufs=6, space="PSUM") as ops:
                    for og in range(2):
                        n_out = 4 if og == 0 else 2
                        pss = [
                            ops.tile([128, 512], F32, tag="po",
                                     name=f"po{_j}")
                            for _j in range(n_out)
                        ]
                        for c in range(FC + 1):
                            kpart = 128 if c < FC else 1
                            wt = load_w(
                                wo_ext, c,
                                slice(og * 512, og * 512 + n_out * 128), kpart,
                            )
                            for j in range(n_out):
                                src = (
                                    av_stack[:, c, :] if c < FC
                                    else ones_row[:, 0:512]
                                )
                                nc.tensor.matmul(
                                    pss[j][0:128, :],
                                    wt[0:kpart, j * 128 : (j + 1) * 128],
                                    src[0:kpart, :] if c < FC else src,
                                    start=(c == 0), stop=(c == FC),
                                )
                        for j in range(n_out):
                            oc = og * 4 + j
                            nc.vector.tensor_add(
                                x2aug[:, oc, tsl], pss[j][:], xaug[:, oc, tsl]
                            )


        # ================ LN2 stats (both batches) + FFN per half =========
        sq2 = arena.tile([128, KC, TOK], F32R, tag="bigB", name="sq2")
        ln_stats(x2aug, rstd2_bc, rstd2_cols, sq2, 0, TOK)

        with (
            tc.tile_pool(name="gsb", bufs=1) as gsb,
            tc.tile_pool(name="fwork", bufs=3) as fwork,
            tc.tile_pool(name="orow", bufs=2) as orow,
            tc.tile_pool(name="fps", bufs=6, space="PSUM") as fps,
            tc.tile_pool(name="trp2", bufs=2, space="PSUM") as trp2,
        ):
            for b in range(BPC):
                hsl = slice(b * 512, (b + 1) * 512)
                g_sb = gsb.tile([128, FFC, 512], F32R, tag="g_sb",
                                name=f"g_sb{b}")
                for og in range(FFC // 4):
                    pss = [fps.tile([128, 512], F32, tag="pf", name=f"pf{_j}")
                           for _j in range(4)]
                    for c in range(KC + 1):
                        kpart = 128 if c < KC else 2
                        wt = load_w(
                            w1_ext, c, slice(og * 512, (og + 1) * 512), kpart,
                        )
                        for j in range(4):
                            nc.tensor.matmul(
                                pss[j][:],
                                wt[0:kpart, j * 128 : (j + 1) * 128],
                                x2aug[0:kpart, min(c, KC), hsl],
                                start=(c == 0), stop=(c == KC),
                            )
                    for j in range(4):
                        pre = fwork.tile([128, 512], F32, tag="pre")
                        nc.vector.tensor_mul(pre[:], pss[j][:], rstd2_bc[:, hsl])
                        nc.scalar.activation(
                            g_sb[:, og * 4 + j, :], pre[:], AF.Gelu
                        )

                # FFN2 + residual -> out (feature-major, fp32)
                out_fm = gsb.tile([128, KC, 512], F32, tag="out_fm",
                                  name=f"out_fm{b}")
                for og in range(2):
                    n_out = 4 if og == 0 else 2
                    pss = [
                        fps.tile([128, 512], F32, tag="pf", name=f"pf2{_j}")
                        for _j in range(n_out)
                    ]
                    for c in range(FFC + 1):
                        kpart = 128 if c < FFC else 1
                        wt = load_w(
                            w2_ext, c,
                            slice(og * 512, og * 512 + n_out * 128), kpart,
                        )
                        for j in range(n_out):
                            src = g_sb[:, c, :] if c < FFC else ones_row[:, hsl]
                            nc.tensor.matmul(
                                pss[j][:],
                                wt[0:kpart, j * 128 : (j + 1) * 128],
                                src,
                                start=(c == 0), stop=(c == FFC),
                            )
                    for j in range(n_out):
                        oc = og * 4 + j
                        nc.vector.tensor_add(
                            out_fm[:, oc, :], pss[j][:], x2aug[:, oc, hsl]
                        )

                # transpose to token-major, one DMA per 128-token row block
                for t in range(4):
                    row = orow.tile([128, D], F32, tag="row")
                    for c in range(KC):
                        pt = trp2.tile([128, 128], F32, tag="tr2")
                        nc.tensor.transpose(
                            pt[:], out_fm[:, c, t * 128 : (t + 1) * 128],
                            ident[:],
                        )
                        nc.scalar.copy(row[:, c * 128 : (c + 1) * 128], pt[:])
                    nc.sync.dma_start(
                        out=out_tiled[:, b * 4 + t, :], in_=row[:]
                    )

    _split_sync_waits(nc, max_waits=1)
    _BUILD_CACHE["nc"] = nc
    return nc


# ---------------------------------------------------------------- kernel()
LAST_EXEC_TIME_NS = None


def kernel(x, params):
    global LAST_EXEC_TIME_NS
    _install_shims()
    from concourse.bass_utils import run_bass_kernel_spmd

    x = np.ascontiguousarray(np.asarray(x, _f32))
    prep = _prep_weights(params)
    nc = _build()

    in_maps = []
    for c in range(N_CORES):
        shard = x[c * BPC : (c + 1) * BPC].reshape(TOK, D)
        in_maps.append({"x": np.ascontiguousarray(shard), **prep})

    trace = bool(int(os.environ.get("KBENCH_TRACE", "0")))
    res = run_bass_kernel_spmd(nc, in_maps, list(range(N_CORES)), trace=trace)
    LAST_EXEC_TIME_NS = res.exec_time_ns

    out = np.empty((B, S, D), _f32)
    for c in range(N_CORES):
        out[c * BPC : (c + 1) * BPC] = res.results[c]["out"].reshape(BPC, S, D)
    return out
